# revision 46
# baseline (speedup 1.0000x reference)
"""Trainium2 Bass kernel for nn_DeepConvGraphEncoderPre.

Model: 4x GCN (dense normalized adjacency) -> mean-pool over nodes ->
single-step BiLSTM -> fc -> temporal attention over T -> linear head.

Sharding: data-parallel over batch B=8 across 8 NeuronCores (1 batch row
per core). edge_index and all weights replicated. The normalized dense
adjacency A^T [256,256] is built ON DEVICE from edge_index via one-hot
matmuls, then every GCN layer is two dense matmuls.

v2 layout (graph-pair packed, bf16):
  activations for a pair of graphs are packed into the full 128-partition
  array:  x_sb[p, k*CW + g*C + c] = x[g, node k*128+p, c]  (CW = 2*C).
  agg  :  lhsT = x slice [128 s, (g,c)|c], rhs = A^T chunk [128 s, 256 d]
          -> psum [(g,c)|c, 256 d]   (accumulate over k)
  Wmm  :  lhsT = agg slice [(g,c), d-chunk], rhs = blockdiag(W,W)
          -> psum [d-chunk, (g,co)]  (L1/L2)
          lhsT = agg_g slice [c, d-chunk], rhs = W -> [d-chunk, co] (L3)
          lhsT = W4 chunk [c-sub, co-sub], rhs = agg_g [c-sub, 256 d]
          -> psum [co-sub, 256 d]    (L4, accumulated over c-sub)
  L4 evac: relu with scale=1/N and accum_out -> pooledT directly.
All GCN matmuls for L2..L4 run in bf16 (FWL hides LDWEIGHTS); L1 runs
fp32r directly on the loaded input data.
"""

import numpy as np

B, T, N, F, E = 8, 32, 256, 64, 4096
H, EMB, OUT = 256, 256, 512
NCORES = 8
NPAIR = T // 2  # graph pairs per core

_CACHE = {}
RUN_KWARGS = {}   # test harness may set {"trace": True, ...}
LAST_RESULT = None


def _build(flags):
    import concourse.mybir as mybir
    import concourse.tile as tile
    from concourse import bacc
    from concourse.masks import make_identity

    dt = mybir.dt
    f32, f32r, bf16, i32 = dt.float32, dt.float32r, dt.bfloat16, dt.int32
    AF = mybir.ActivationFunctionType
    ALU = mybir.AluOpType

    gcn_bias, lstm_bias, fc_bias, out_bias = (
        flags["gcn_bias"], flags["lstm_bias"], flags["fc_bias"], flags["out_bias"])

    nc = bacc.Bacc("TRN2", target_bir_lowering=False, debug=False,
                   num_devices=NCORES)

    def r(ap):
        return ap.bitcast(f32r)

    def rf(ap):
        return ap.bitcast(f32)

    # ---------------- DRAM I/O ----------------
    data_d = nc.dram_tensor("data_local", [T, N, F], f32r, kind="ExternalInput")
    edge_d = nc.dram_tensor("edge_index", [2, E], i32, kind="ExternalInput")
    W_d = [nc.dram_tensor(f"W{i+1}", [c_in, c_out], f32, kind="ExternalInput")
           for i, (c_in, c_out) in enumerate([(64, 64), (64, 128), (128, 256), (256, 256)])]
    Wih_d = {d: nc.dram_tensor(f"W_ih_{d}", [4 * H, H], f32, kind="ExternalInput")
             for d in ("f", "b")}
    fcW_d = nc.dram_tensor("fc_W", [2 * H, EMB], f32r, kind="ExternalInput")
    attnW_d = nc.dram_tensor("attn_W", [EMB, 1], f32r, kind="ExternalInput")
    outW_d = nc.dram_tensor("out_W", [EMB, OUT], f32r, kind="ExternalInput")
    if gcn_bias:
        b_d = [nc.dram_tensor(f"b{i+1}", [c], f32, kind="ExternalInput")
               for i, c in enumerate([64, 128, 256, 256])]
    if lstm_bias:
        bih_d = {d: nc.dram_tensor(f"b_ih_{d}", [4 * H], f32, kind="ExternalInput")
                 for d in ("f", "b")}
        bhh_d = {d: nc.dram_tensor(f"b_hh_{d}", [4 * H], f32, kind="ExternalInput")
                 for d in ("f", "b")}
    if fc_bias:
        fcb_d = nc.dram_tensor("fc_b", [EMB], f32r, kind="ExternalInput")
    if out_bias:
        outb_d = nc.dram_tensor("out_b", [OUT], f32r, kind="ExternalInput")
    out_d = nc.dram_tensor("out", [1, OUT], f32, kind="ExternalOutput")

    with tile.TileContext(nc) as tc:
        # ================= persistent constants =================
        with tc.tile_pool(name="const", bufs=1) as cp:
            AT_r = cp.tile([128, 512], f32r)           # col k*256+d ; A^T[s,d], s=k*128+p
            AT_b = cp.tile([128, 512], bf16)           # same, bf16
            W1blk = cp.tile([128, 128], bf16)          # blockdiag(W1, W1)
            W2blk = cp.tile([128, 256], bf16)          # blockdiag(W2, W2)
            W3_b = cp.tile([128, 256], bf16)           # W3 natural
            W4_b = cp.tile([128, 512], bf16)           # col cc*256 + co
            WihT_sb = [cp.tile([128, 2048], bf16, name=f"WihT{k}") for k in (0, 1)]
            pooledT_bf = cp.tile([128, 64], bf16)
            fcW_sb = cp.tile([128, 1024], f32r)        # col k*256+m
            fcW_bf = cp.tile([128, 1024], bf16)        # bf16 copy for tail mms
            outW_sb = cp.tile([128, 1024], f32r)       # col m*512+o
            outW_bf = cp.tile([128, 1024], bf16)
            attnW_sb = cp.tile([128, 2], f32r)         # col m
            attnW_bf = cp.tile([128, 2], bf16)
            ones_col = cp.tile([128, 1], f32r)
            ones_11 = cp.tile([1, 1], f32r)
            ident = cp.tile([128, 128], f32)
            pooledT_sb = cp.tile([128, 64], f32r)      # col mo*32 + t ; sum relu(z4)/N
            x0_all = cp.tile([128, NPAIR * 256], f32r)  # col j*256+k*128+g*64+c
            x1_all = cp.tile([128, NPAIR * 256], bf16)  # col j*256+m*128+g*64+co
            x2_all = cp.tile([128, NPAIR * 512], bf16)  # col j*512+m*256+g*128+co
            x3_all = cp.tile([128, T * 512], bf16)      # col t*512+m*256+co
            if gcn_bias:
                bb1 = cp.tile([128, 128], bf16)        # col g*64+c  -> b1[c]
                bb2 = cp.tile([128, 256], bf16)        # col g*128+c -> b2[c]
                bb3 = cp.tile([128, 256], bf16)        # col c      -> b3[c]
                b4col_sb = cp.tile([128, 2], f32)      # scaled by 1/N
            if lstm_bias:
                lbias_sb = cp.tile([1, 2048], f32r)    # col gate*512+dir*256+h
            if fc_bias:
                fcb_row = cp.tile([1, EMB], f32r)
                fcb_col = cp.tile([128, 2], f32)
            if out_bias:
                outb_row = cp.tile([1, OUT], f32r)

            ones_f = cp.tile([128, 1], f32)
            scr11 = cp.tile([1, 1], f32)
            nc.gpsimd.memset(ones_f[:], 1.0)
            nc.vector.tensor_copy(ones_col[:], ones_f[:])
            nc.vector.tensor_copy(ones_11[:], ones_f[0:1, :])
            make_identity(nc, ident[:])
            # pre-warm the ACT sqrt table while DMAs are in flight
            nc.scalar.sqrt(scr11[:], ones_f[0:1, :])

            # ---- edge DMA first (A-build is the critical path) ----
            eg_i = cp.tile([128, 64], i32)   # col j<32: src ; col 32+j: dst
            nc.sync.dma_start(
                out=eg_i[:].rearrange("p (two j) -> p two j", two=2),
                in_=edge_d.ap().rearrange("two (p j) -> p two j", p=128))

            # ---- input prefetch ----
            dv = data_d.ap().rearrange("(j two) (k p) c -> two k p j c", two=2, p=128)
            xv = x0_all[:].rearrange("p (j k g c) -> k g p j c", j=NPAIR, k=2, g=2)
            for k in (0, 1):
                for g in (0, 1):
                    nc.sync.dma_start(out=xv[k, g], in_=dv[g, k])

            # ---- weight loads (natural layouts) ----
            w1s = cp.tile([64, 64], f32)
            w2s = cp.tile([64, 128], f32)
            w3s = cp.tile([128, 256], f32)
            w4s = cp.tile([128, 512], f32)
            wnat = [cp.tile([128, 2048], f32, name=f"wnat{k}") for k in (0, 1)]
            nc.sync.dma_start(out=w1s[:], in_=W_d[0].ap())
            nc.sync.dma_start(out=w2s[:], in_=W_d[1].ap())
            nc.sync.dma_start(out=w3s[:], in_=W_d[2].ap())
            nc.sync.dma_start(
                out=w4s[:].rearrange("p (k co) -> p k co", k=2),
                in_=W_d[3].ap().rearrange("(k p) co -> p k co", p=128))
            for di, d in enumerate(("f", "b")):
                nc.sync.dma_start(
                    out=wnat[di][:].rearrange("p (i c) -> p i c", i=8),
                    in_=Wih_d[d].ap().rearrange("(i p) c -> p i c", p=128))
            nc.sync.dma_start(
                out=fcW_sb[:].rearrange("p (k m) -> p k m", k=4),
                in_=fcW_d.ap().rearrange("(k p) m -> p k m", p=128))
            nc.sync.dma_start(
                out=outW_sb[:].rearrange("p (m o) -> p m o", m=2),
                in_=outW_d.ap().rearrange("(m p) o -> p m o", p=128))
            nc.sync.dma_start(
                out=attnW_sb[:].unsqueeze(2),
                in_=attnW_d.ap().rearrange("(m p) one -> p m one", p=128))
            if fc_bias:
                nc.sync.dma_start(out=fcb_row[:], in_=fcb_d.ap().rearrange("m -> 1 m"))
                nc.sync.dma_start(
                    out=fcb_col[:].unsqueeze(2),
                    in_=fcb_d.ap().rearrange("(m p) -> p m", p=128).unsqueeze(2))
            if out_bias:
                nc.sync.dma_start(out=outb_row[:], in_=outb_d.ap().rearrange("o -> 1 o"))

            # blockdiag weight prep on ACT (keep DVE free for one-hots)
            nc.gpsimd.memset(W1blk[:], 0.0)
            nc.gpsimd.memset(W2blk[:], 0.0)
            for g in (0, 1):
                nc.scalar.copy(W1blk[g * 64:(g + 1) * 64, g * 64:(g + 1) * 64],
                               w1s[:])
                nc.scalar.copy(W2blk[g * 64:(g + 1) * 64, g * 128:(g + 1) * 128],
                               w2s[:])
            nc.scalar.copy(W3_b[:], w3s[:])
            nc.scalar.copy(W4_b[:], w4s[:])
            nc.scalar.copy(fcW_bf[:], fcW_sb[:].bitcast(f32))
            nc.scalar.copy(attnW_bf[:], attnW_sb[:].bitcast(f32))
            nc.scalar.copy(outW_bf[:], outW_sb[:].bitcast(f32))

            # ============ stage 0: A^T build ============
            with (
                tc.tile_pool(name="ab_sb", bufs=2) as ab,
                tc.tile_pool(name="ab_ps", bufs=1, space="PSUM") as abp,
                tc.tile_pool(name="oh", bufs=6) as ohp,
            ):
                # iota 0..255 on all 128 partitions directly (no broadcast dep)
                iota_i = ab.tile([128, 256], i32)
                nc.gpsimd.iota(iota_i[:], pattern=[[1, 256]], base=0,
                               channel_multiplier=0)
                iota_bc = ab.tile([128, 256], bf16)
                nc.vector.tensor_copy(iota_bc[:], iota_i[:])

                # edge index columns (DMA'd into eg_i early), cast to f32
                eg_b = ab.tile([128, 64], f32)
                nc.vector.tensor_copy(eg_b[:], eg_i[:])

                # self-loop columns p+128k
                sl_i = ab.tile([128, 2], i32)
                nc.gpsimd.iota(sl_i[:], pattern=[[128, 2]], base=0,
                               channel_multiplier=1)
                sl_b = ab.tile([128, 2], f32)
                nc.vector.tensor_copy(sl_b[:], sl_i[:])

                # accumulate A^T_unnorm = sum_e onehot_src^T(slice) @ onehot_dst
                atun_ps = abp.tile([128, 512], f32)
                for c in range(34):
                    if c < 32:
                        scol = eg_b[:, c:c + 1]
                        dcol = eg_b[:, 32 + c:33 + c]
                    else:
                        scol = dcol = sl_b[:, c - 32:c - 31]
                    oh_s = ohp.tile([128, 256], bf16, tag="ohs")
                    nc.vector.tensor_scalar(oh_s[:], iota_bc[:], scol, None,
                                            op0=ALU.is_equal)
                    if c < 32:
                        oh_d = ohp.tile([128, 256], bf16, tag="ohd")
                        nc.vector.tensor_scalar(oh_d[:], iota_bc[:], dcol, None,
                                                op0=ALU.is_equal)
                    else:
                        oh_d = oh_s
                    for m in (0, 1):
                        nc.tensor.matmul(atun_ps[:, m * 256:(m + 1) * 256],
                                         oh_s[:, m * 128:(m + 1) * 128], oh_d[:],
                                         start=(c == 0 and m == 0),
                                         stop=(c == 33 and m == 1))
                atun_sb = ab.tile([128, 512], f32r)
                nc.scalar.copy(atun_sb[:], atun_ps[:])

                # deg (in-degree, row form), dinv = 1/sqrt(deg)  (deg >= 1 always)
                deg_ps = abp.tile([1, 256], f32, tag="deg")
                for m in (0, 1):
                    nc.tensor.matmul(deg_ps[:], rf(ones_col[:]),
                                     rf(atun_sb[:, m * 256:(m + 1) * 256]),
                                     start=(m == 0), stop=(m == 1))
                deg_sb = ab.tile([1, 256], f32)
                nc.scalar.copy(deg_sb[:], deg_ps[:])
                # column form via PE transpose of the deg row; rsqrt on [128,2]
                degc_ps = abp.tile([128, 2], f32, tag="degc")
                for dm in (0, 1):
                    nc.tensor.transpose(degc_ps[:, dm:dm + 1],
                                        deg_sb[0:1, dm * 128:(dm + 1) * 128],
                                        ident[0:1, 0:1])
                dinv_col = ab.tile([128, 2], f32)
                nc.vector.reciprocal(dinv_col[:], degc_ps[:])
                nc.scalar.sqrt(dinv_col[:], dinv_col[:])
                # row form back via transposes of dinv_col, then broadcast
                dinvr_ps = abp.tile([1, 256], f32, tag="dinvr")
                for dm in (0, 1):
                    nc.tensor.transpose(dinvr_ps[0:1, dm * 128:(dm + 1) * 128],
                                        dinv_col[:, dm:dm + 1],
                                        ident[:])
                dinvr_sb = ab.tile([1, 256], f32)
                nc.scalar.copy(dinvr_sb[:], dinvr_ps[:])
                dinv_bc = ab.tile([128, 256], f32)
                nc.gpsimd.partition_broadcast(dinv_bc[:], dinvr_sb[:])

                # AT_norm[s,d] = dinv[s] * ATun[s,d] * dinv[d]
                for m in (0, 1):
                    nc.vector.scalar_tensor_tensor(
                        out=AT_r[:, m * 256:(m + 1) * 256],
                        in0=atun_sb[:, m * 256:(m + 1) * 256],
                        scalar=dinv_col[:, m:m + 1],
                        in1=dinv_bc[:],
                        op0=ALU.mult, op1=ALU.mult)
                nc.scalar.copy(AT_b[:], rf(AT_r[:]))

                # ---- bias prep ----
                if gcn_bias:
                    b1r = ab.tile([1, 64], f32, tag="b1r")
                    nc.sync.dma_start(out=b1r[:], in_=b_d[0].ap().rearrange("c -> 1 c"))
                    brow = ab.tile([1, 128], f32, tag="brow")
                    nc.vector.tensor_copy(
                        brow[:].rearrange("one (r c) -> one r c", r=2),
                        b1r[:].rearrange("one c -> one 1 c").broadcast_to([1, 2, 64]))
                    nc.gpsimd.partition_broadcast(bb1[:], brow[:])
                    b2r = ab.tile([1, 128], f32, tag="b2r")
                    nc.sync.dma_start(out=b2r[:], in_=b_d[1].ap().rearrange("c -> 1 c"))
                    brow2 = ab.tile([1, 256], f32, tag="brow2")
                    nc.vector.tensor_copy(
                        brow2[:].rearrange("one (r c) -> one r c", r=2),
                        b2r[:].rearrange("one c -> one 1 c").broadcast_to([1, 2, 128]))
                    nc.gpsimd.partition_broadcast(bb2[:], brow2[:])
                    brow3 = ab.tile([1, 256], f32, tag="brow3")
                    nc.sync.dma_start(out=brow3[:],
                                      in_=b_d[2].ap().rearrange("c -> 1 c"))
                    nc.gpsimd.partition_broadcast(bb3[:], brow3[:])
                    b4tmp = ab.tile([128, 2], f32, tag="b4tmp")
                    nc.sync.dma_start(
                        out=b4tmp[:].unsqueeze(2),
                        in_=b_d[3].ap().rearrange("(m p) -> p m", p=128).unsqueeze(2))
                    nc.vector.tensor_scalar_mul(b4col_sb[:], b4tmp[:], 1.0 / N)
                if lstm_bias:
                    for di, d in enumerate(("f", "b")):
                        bi = ab.tile([1, 1024], f32, tag="lbias_i")
                        bh = ab.tile([1, 1024], f32, tag="lbias_h")
                        nc.sync.dma_start(out=bi[:], in_=bih_d[d].ap().rearrange("g -> 1 g"))
                        nc.sync.dma_start(out=bh[:], in_=bhh_d[d].ap().rearrange("g -> 1 g"))
                        nc.vector.tensor_add(
                            lbias_sb[:, di * 256:].rearrange("one (g q) -> one g q", g=4)[:, :, 0:256],
                            bi[:].rearrange("one (g q) -> one g q", g=4),
                            bh[:].rearrange("one (g q) -> one g q", g=4))

            # ================= main GCN loop (layer sweeps) =================
            with (
                tc.tile_pool(name="work", bufs=4) as wk,
                tc.tile_pool(name="ps", bufs=1, space="PSUM") as ps,
            ):
                # ---- L1 sweep (fp32r) ----
                for j in range(NPAIR):
                    xb = j * 256
                    agg1 = ps.tile([128, 256], f32, tag="agg", bufs=3)
                    for k in (0, 1):
                        nc.tensor.matmul(
                            agg1[:], x0_all[:, xb + k * 128: xb + (k + 1) * 128],
                            AT_r[:, k * 256:(k + 1) * 256],
                            start=(k == 0), stop=(k == 1))
                    agg1_sb = wk.tile([128, 256], bf16, tag="agg1sb")
                    nc.scalar.copy(agg1_sb[:], agg1[:])
                    z1full = ps.tile([128, 512], f32, tag="z", bufs=3)
                    for m in (0, 1):
                        nc.tensor.matmul(
                            z1full[:, m * 128:(m + 1) * 128],
                            agg1_sb[:, m * 128:(m + 1) * 128], W1blk[:],
                            start=True, stop=True)
                    x1v = x1_all[:, xb:xb + 256]
                    if gcn_bias:
                        zt = wk.tile([128, 256], bf16, tag="zt1")
                        nc.vector.tensor_add(
                            zt[:].rearrange("p (m q) -> p m q", m=2),
                            z1full[:, 0:256].rearrange("p (m q) -> p m q", m=2),
                            bb1[:].rearrange("p q -> p 1 q").broadcast_to([128, 2, 128]))
                        nc.vector.tensor_relu(x1v, zt[:])
                    else:
                        nc.vector.tensor_relu(x1v, z1full[:, 0:256])

                # ---- W_ih transpose (fills the dinv-chain PE gap) ----
                for di in (0, 1):
                    for ks in (0, 1):
                        wt_ps = ps.tile([128, 512], f32, tag="z", bufs=3)
                        wt_ps2 = ps.tile([128, 512], f32, tag="z", bufs=3)
                        for i in range(8):
                            tgt = wt_ps if i < 4 else wt_ps2
                            nc.tensor.transpose(
                                tgt[:, (i % 4) * 128:(i % 4 + 1) * 128],
                                wnat[di][:, i * 256 + ks * 128:
                                          i * 256 + (ks + 1) * 128],
                                ident[:])
                        # evac with (gate,hh) regrouping; no 1/N (folded in L4)
                        dst = WihT_sb[ks][:].rearrange(
                            "p (g d hh r) -> p g d hh r", g=4, d=2, hh=2)[:, :, di]
                        src1 = wt_ps[:].rearrange("p (g hh r) -> p g hh r",
                                                  g=2, hh=2)
                        src2 = wt_ps2[:].rearrange("p (g hh r) -> p g hh r",
                                                   g=2, hh=2)
                        if di == 0:
                            nc.vector.tensor_copy(dst[:, 0:2], src1)
                            nc.vector.tensor_copy(dst[:, 2:4], src2)
                        else:
                            nc.scalar.copy(dst[:, 0:2], src1)
                            nc.scalar.copy(dst[:, 2:4], src2)

                # ---- L2 sweep (bf16) ----
                for j in range(NPAIR):
                    xb = j * 256
                    agg2 = ps.tile([128, 256], f32, tag="agg", bufs=3)
                    for k in (0, 1):
                        nc.tensor.matmul(
                            agg2[:], x1_all[:, xb + k * 128: xb + (k + 1) * 128],
                            AT_b[:, k * 256:(k + 1) * 256],
                            start=(k == 0), stop=(k == 1))
                    agg2_sb = wk.tile([128, 256], bf16, tag="agg2sb")
                    nc.scalar.copy(agg2_sb[:], agg2[:])
                    z2 = ps.tile([128, 512], f32, tag="z", bufs=3)
                    for m in (0, 1):
                        nc.tensor.matmul(
                            z2[:, m * 256:(m + 1) * 256],
                            agg2_sb[:, m * 128:(m + 1) * 128], W2blk[:],
                            start=True, stop=True)
                    x2v = x2_all[:, j * 512:(j + 1) * 512]
                    if gcn_bias:
                        zt = wk.tile([128, 512], bf16, tag="zt2")
                        nc.vector.tensor_add(
                            zt[:].rearrange("p (m q) -> p m q", m=2),
                            z2[:].rearrange("p (m q) -> p m q", m=2),
                            bb2[:].rearrange("p q -> p 1 q").broadcast_to([128, 2, 256]))
                        nc.vector.tensor_relu(x2v, zt[:])
                    else:
                        nc.vector.tensor_relu(x2v, z2[:])

                # ---- L3 sweep (bf16, per-graph) ----
                for j in range(NPAIR):
                    for g in (0, 1):
                        agg3 = ps.tile([128, 256], f32, tag="agg", bufs=3)
                        for k in (0, 1):
                            nc.tensor.matmul(
                                agg3[:],
                                x2_all[:, j * 512 + k * 256 + g * 128:
                                       j * 512 + k * 256 + (g + 1) * 128],
                                AT_b[:, k * 256:(k + 1) * 256],
                                start=(k == 0), stop=(k == 1))
                        agg3_sb = wk.tile([128, 256], bf16, tag="agg3sb")
                        nc.scalar.copy(agg3_sb[:], agg3[:])
                        z3 = ps.tile([128, 512], f32, tag="z", bufs=3)
                        for m in (0, 1):
                            nc.tensor.matmul(
                                z3[:, m * 256:(m + 1) * 256],
                                agg3_sb[:, m * 128:(m + 1) * 128], W3_b[:],
                                start=True, stop=True)
                        t_idx = 2 * j + g
                        x3v = x3_all[:, t_idx * 512:(t_idx + 1) * 512]
                        if gcn_bias:
                            zt = wk.tile([128, 512], bf16, tag="zt3")
                            nc.vector.tensor_add(
                                zt[:].rearrange("p (m q) -> p m q", m=2),
                                z3[:].rearrange("p (m q) -> p m q", m=2),
                                bb3[:].rearrange("p q -> p 1 q").broadcast_to([128, 2, 256]))
                            nc.vector.tensor_relu(x3v, zt[:])
                        else:
                            nc.vector.tensor_relu(x3v, z3[:])

                # ---- L4 sweep (bf16, per-graph; relu/N + accum -> pooledT) ----
                for j in range(NPAIR):
                    for g in (0, 1):
                        t_idx = 2 * j + g
                        xb3 = t_idx * 512
                        agg4_sb = wk.tile([128, 512], bf16, tag="agg4sb")
                        agg4 = ps.tile([128, 512], f32, tag="z", bufs=3)
                        for cc in (0, 1):
                            for k in (0, 1):
                                nc.tensor.matmul(
                                    agg4[:, cc * 256:(cc + 1) * 256],
                                    x3_all[:, xb3 + k * 256 + cc * 128:
                                           xb3 + k * 256 + (cc + 1) * 128],
                                    AT_b[:, k * 256:(k + 1) * 256],
                                    start=(k == 0), stop=(k == 1))
                        nc.vector.tensor_copy(agg4_sb[:], agg4[:])
                        for mo in (0, 1):
                            z4 = ps.tile([128, 256], f32, tag="z4", bufs=2)
                            for cc in (0, 1):
                                nc.tensor.matmul(
                                    z4[:],
                                    W4_b[:, cc * 256 + mo * 128: cc * 256 + (mo + 1) * 128],
                                    agg4_sb[:, cc * 256:(cc + 1) * 256],
                                    start=(cc == 0), stop=(cc == 1))
                            x4scr = wk.tile([128, 256], bf16, tag="x4scr")
                            ctx_lp = nc.allow_low_precision(
                                reason="fp32r accum (32-bit)")
                            ctx_lp.__enter__()
                            nc.scalar.activation(
                                x4scr[:], z4[:], AF.Relu, scale=1.0 / N,
                                bias=(b4col_sb[:, mo:mo + 1] if gcn_bias
                                      else 0.0),
                                accum_out=pooledT_sb[:, mo * 32 + t_idx:
                                                     mo * 32 + t_idx + 1])
                            ctx_lp.__exit__(None, None, None)

                # pre-warm the ACT tanh table before the tail needs it
                nc.scalar.activation(scr11[:], ones_f[0:1, :], AF.Tanh)

            # ================= LSTM + fc + attention + head =================
            with (
                tc.tile_pool(name="tail", bufs=1) as tl,
                tc.tile_pool(name="tailps_g", bufs=1, space="PSUM") as tpg,
                tc.tile_pool(name="tailps", bufs=2, space="PSUM") as tp,
            ):
                # gates [32, 2048] col gate*512 + dir*256 + h  (pooled already /N)
                g_ps = tpg.tile([32, 2048], f32, tag="gates")
                if lstm_bias:
                    ones_r32 = tl.tile([1, 32], f32r)
                    ones_r32f = tl.tile([1, 32], f32)
                    nc.gpsimd.memset(ones_r32f[:], 1.0)
                    nc.vector.tensor_copy(ones_r32[:], ones_r32f[:])
                nc.vector.tensor_copy(pooledT_bf[:], rf(pooledT_sb[:]))
                for s in range(4):
                    for k in (0, 1):
                        nc.tensor.matmul(
                            g_ps[:, s * 512:(s + 1) * 512],
                            pooledT_bf[:, k * 32:(k + 1) * 32],
                            WihT_sb[k][:, s * 512:(s + 1) * 512],
                            start=(k == 0), stop=(k == 1 and not lstm_bias))
                    if lstm_bias:
                        nc.tensor.matmul(g_ps[:, s * 512:(s + 1) * 512],
                                         r(ones_r32[:]),
                                         r(lbias_sb[:, s * 512:(s + 1) * 512]),
                                         start=False, stop=True)
                # sigmoid(x) = 0.5*tanh(x/2)+0.5 -> single Tanh table load
                tanh_g = tl.tile([32, 512], f32)
                th_i = tl.tile([32, 512], f32)
                th_o = tl.tile([32, 512], f32)
                nc.scalar.activation(tanh_g[:], g_ps[:, 1024:1536], AF.Tanh)
                nc.scalar.activation(th_i[:], g_ps[:, 0:512], AF.Tanh, scale=0.5)
                nc.scalar.activation(th_o[:], g_ps[:, 1536:2048], AF.Tanh, scale=0.5)
                sig_i = tl.tile([32, 512], f32)
                sig_o = tl.tile([32, 512], f32)
                nc.vector.tensor_scalar(sig_i[:], th_i[:], 0.5, 0.5,
                                        op0=ALU.mult, op1=ALU.add)
                nc.vector.tensor_scalar(sig_o[:], th_o[:], 0.5, 0.5,
                                        op0=ALU.mult, op1=ALU.add)
                c_sb = tl.tile([32, 512], f32)
                nc.vector.tensor_mul(c_sb[:], sig_i[:], tanh_g[:])
                tc_sb = tl.tile([32, 512], f32)
                nc.scalar.activation(tc_sb[:], c_sb[:], AF.Tanh)
                h_sb = tl.tile([32, 512], f32)
                nc.vector.tensor_mul(h_sb[:], sig_o[:], tc_sb[:])

                # transpose h -> hT [128, (k,t)]  (bf16 for the small tail mms)
                hT_ps = tp.tile([128, 128], f32, tag="small")
                for k in range(4):
                    nc.tensor.transpose(hT_ps[:, k * 32:(k + 1) * 32],
                                        h_sb[:, k * 128:(k + 1) * 128],
                                        ident[0:32, 0:32])
                hT_sb = tl.tile([128, 128], bf16)
                nc.vector.tensor_copy(hT_sb[:], hT_ps[:])

                # emb (node-major) [32, 256]
                emb_ps = tp.tile([32, 256], f32, tag="small")
                for k in range(4):
                    nc.tensor.matmul(emb_ps[:], hT_sb[:, k * 32:(k + 1) * 32],
                                     fcW_bf[:, k * 256:(k + 1) * 256],
                                     start=(k == 0), stop=(k == 3))
                if fc_bias:
                    fcb_bc = tl.tile([32, 256], f32)
                    nc.gpsimd.partition_broadcast(fcb_bc[:], rf(fcb_row[:]))
                    nc.vector.tensor_add(emb_ps[:], emb_ps[:], fcb_bc[:])
                emb_sb = tl.tile([32, 256], bf16)
                nc.vector.tensor_copy(emb_sb[:], emb_ps[:])

                # embT [128, (mo,t)]
                embT_ps = tp.tile([128, 64], f32, tag="small")
                for mo in (0, 1):
                    for k in range(4):
                        nc.tensor.matmul(
                            embT_ps[:, mo * 32:(mo + 1) * 32],
                            fcW_bf[:, k * 256 + mo * 128: k * 256 + (mo + 1) * 128],
                            hT_sb[:, k * 32:(k + 1) * 32],
                            start=(k == 0), stop=(k == 3))
                embT_sb = tl.tile([128, 64], bf16)
                if fc_bias:
                    for mo in (0, 1):
                        nc.scalar.activation(embT_sb[:, mo * 32:(mo + 1) * 32],
                                             embT_ps[:, mo * 32:(mo + 1) * 32],
                                             AF.Identity,
                                             bias=fcb_col[:, mo:mo + 1])
                else:
                    nc.vector.tensor_copy(embT_sb[:], embT_ps[:])

                # attention scores [1, 32] ; softmax over free dim
                sc_ps = tp.tile([1, 32], f32, tag="small")
                for mo in (0, 1):
                    nc.tensor.matmul(sc_ps[:], attnW_bf[:, mo:mo + 1],
                                     embT_sb[:, mo * 32:(mo + 1) * 32],
                                     start=(mo == 0), stop=(mo == 1))
                sc_sb = tl.tile([1, 32], f32)
                nc.vector.tensor_copy(sc_sb[:], sc_ps[:])
                mx = tl.tile([1, 1], f32)
                nc.vector.tensor_reduce(mx[:], sc_sb[:], axis=mybir.AxisListType.X,
                                        op=ALU.max)
                mxn = tl.tile([1, 1], f32)
                nc.vector.tensor_scalar_mul(mxn[:], mx[:], -1.0)
                ex = tl.tile([1, 32], f32)
                ssum = tl.tile([1, 1], f32)
                nc.scalar.activation(ex[:], sc_sb[:], AF.Exp, bias=mxn[:],
                                     accum_out=ssum[:])
                rs = tl.tile([1, 1], f32)
                nc.vector.reciprocal(rs[:], ssum[:])
                w_row = tl.tile([1, 32], f32r)
                nc.vector.tensor_scalar_mul(w_row[:], ex[:], rs[:])

                # w column; xwc[m,1] = sum_t emb[t,m] * w[t]  (direct, no xw row)
                wc_ps = tp.tile([32, 1], f32, tag="small")
                nc.tensor.matmul(wc_ps[:], rf(w_row[:]), rf(ones_11[:]),
                                 start=True, stop=True)
                wc_sb = tl.tile([32, 1], bf16)
                nc.vector.tensor_copy(wc_sb[:], wc_ps[:])
                xwc_ps = tp.tile([128, 2], f32, tag="small")
                for mo in (0, 1):
                    nc.tensor.matmul(xwc_ps[:, mo:mo + 1],
                                     emb_sb[:, mo * 128:(mo + 1) * 128],
                                     wc_sb[:], start=True, stop=True)
                xwc_bf = tl.tile([128, 2], bf16)
                nc.vector.tensor_copy(xwc_bf[:], xwc_ps[:])
                fin_ps = tp.tile([1, 512], f32, tag="small")
                for mo in (0, 1):
                    nc.tensor.matmul(fin_ps[:], xwc_bf[:, mo:mo + 1],
                                     outW_bf[:, mo * 512:(mo + 1) * 512],
                                     start=(mo == 0), stop=(mo == 1 and not out_bias))
                if out_bias:
                    nc.tensor.matmul(fin_ps[:], rf(ones_11[:]), rf(outb_row[:]),
                                     start=False, stop=True)
                fin_sb = tl.tile([1, 512], f32)
                nc.vector.tensor_copy(fin_sb[:], fin_ps[:])
                nc.sync.dma_start(out=out_d.ap(), in_=fin_sb[:])

    nc.compile()
    return nc


def _get_nc(flags):
    key = tuple(sorted(flags.items()))
    if key not in _CACHE:
        _CACHE[key] = _build(flags)
    return _CACHE[key]


def kernel(**inputs):
    from concourse import bass_utils

    inp = {k: np.asarray(v) for k, v in inputs.items()}
    flags = {
        "gcn_bias": any(np.any(inp[f"b{i}"]) for i in (1, 2, 3, 4)),
        "lstm_bias": any(np.any(inp[k]) for k in
                         ("b_ih_f", "b_hh_f", "b_ih_b", "b_hh_b")),
        "fc_bias": bool(np.any(inp["fc_b"])),
        "out_bias": bool(np.any(inp["out_b"])),
    }
    nc = _get_nc(flags)

    base = {
        "edge_index": np.ascontiguousarray(inp["edge_index"].astype(np.int32)),
        "W1": np.ascontiguousarray(inp["W1"].astype(np.float32)),
        "W2": np.ascontiguousarray(inp["W2"].astype(np.float32)),
        "W3": np.ascontiguousarray(inp["W3"].astype(np.float32)),
        "W4": np.ascontiguousarray(inp["W4"].astype(np.float32)),
        "W_ih_f": np.ascontiguousarray(inp["W_ih_f"].astype(np.float32)),
        "W_ih_b": np.ascontiguousarray(inp["W_ih_b"].astype(np.float32)),
        "fc_W": np.ascontiguousarray(inp["fc_W"].astype(np.float32)),
        "attn_W": np.ascontiguousarray(inp["attn_W"].astype(np.float32)),
        "out_W": np.ascontiguousarray(inp["out_W"].astype(np.float32)),
    }
    if flags["gcn_bias"]:
        for i in (1, 2, 3, 4):
            base[f"b{i}"] = np.ascontiguousarray(inp[f"b{i}"].astype(np.float32))
    if flags["lstm_bias"]:
        for k in ("b_ih_f", "b_hh_f", "b_ih_b", "b_hh_b"):
            base[k] = np.ascontiguousarray(inp[k].astype(np.float32))
    if flags["fc_bias"]:
        base["fc_b"] = np.ascontiguousarray(inp["fc_b"].astype(np.float32))
    if flags["out_bias"]:
        base["out_b"] = np.ascontiguousarray(inp["out_b"].astype(np.float32))

    data = inp["data"].astype(np.float32)
    in_maps = [dict(base, data_local=np.ascontiguousarray(data[c]))
               for c in range(NCORES)]

    global LAST_RESULT
    res = bass_utils.run_bass_kernel_spmd(nc, in_maps,
                                          core_ids=list(range(NCORES)),
                                          **RUN_KWARGS)
    LAST_RESULT = res
    return np.concatenate([r["out"] for r in res.results], axis=0)


if __name__ == "__main__":
    import reference
    inputs = {k: np.asarray(v) for k, v in reference.setup_inputs().items()}
    got = kernel(**inputs)
    print(got.shape, got.dtype)


# revision 50
# speedup vs baseline: 1.0383x; 1.0383x over previous
"""Trainium2 Bass kernel for nn_DeepConvGraphEncoderPre.

Model: 4x GCN (dense normalized adjacency) -> mean-pool over nodes ->
single-step BiLSTM -> fc -> temporal attention over T -> linear head.

Sharding: data-parallel over batch B=8 across 8 NeuronCores (1 batch row
per core). edge_index and all weights replicated. The normalized dense
adjacency A^T [256,256] is built ON DEVICE from edge_index via one-hot
matmuls, then every GCN layer is two dense matmuls.

v2 layout (graph-pair packed, bf16):
  activations for a pair of graphs are packed into the full 128-partition
  array:  x_sb[p, k*CW + g*C + c] = x[g, node k*128+p, c]  (CW = 2*C).
  agg  :  lhsT = x slice [128 s, (g,c)|c], rhs = A^T chunk [128 s, 256 d]
          -> psum [(g,c)|c, 256 d]   (accumulate over k)
  Wmm  :  lhsT = agg slice [(g,c), d-chunk], rhs = blockdiag(W,W)
          -> psum [d-chunk, (g,co)]  (L1/L2)
          lhsT = agg_g slice [c, d-chunk], rhs = W -> [d-chunk, co] (L3)
          lhsT = W4 chunk [c-sub, co-sub], rhs = agg_g [c-sub, 256 d]
          -> psum [co-sub, 256 d]    (L4, accumulated over c-sub)
  L4 evac: relu with scale=1/N and accum_out -> pooledT directly.
All GCN matmuls for L2..L4 run in bf16 (FWL hides LDWEIGHTS); L1 runs
fp32r directly on the loaded input data.
"""

import numpy as np

B, T, N, F, E = 8, 32, 256, 64, 4096
H, EMB, OUT = 256, 256, 512
NCORES = 8
NPAIR = T // 2  # graph pairs per core

_CACHE = {}
RUN_KWARGS = {}   # test harness may set {"trace": True, ...}
LAST_RESULT = None


def _build(flags):
    import concourse.mybir as mybir
    import concourse.tile as tile
    from concourse import bacc
    from concourse.masks import make_identity

    dt = mybir.dt
    f32, f32r, bf16, i32 = dt.float32, dt.float32r, dt.bfloat16, dt.int32
    AF = mybir.ActivationFunctionType
    ALU = mybir.AluOpType

    gcn_bias, lstm_bias, fc_bias, out_bias = (
        flags["gcn_bias"], flags["lstm_bias"], flags["fc_bias"], flags["out_bias"])

    nc = bacc.Bacc("TRN2", target_bir_lowering=False, debug=False,
                   num_devices=NCORES)

    def r(ap):
        return ap.bitcast(f32r)

    def rf(ap):
        return ap.bitcast(f32)

    # ---------------- DRAM I/O ----------------
    data_d = nc.dram_tensor("data_local", [T, N, F], f32r, kind="ExternalInput")
    edge_d = nc.dram_tensor("edge_index", [2, E], i32, kind="ExternalInput")
    W_d = [nc.dram_tensor(f"W{i+1}", [c_in, c_out], f32, kind="ExternalInput")
           for i, (c_in, c_out) in enumerate([(64, 64), (64, 128), (128, 256), (256, 256)])]
    Wih_d = {d: nc.dram_tensor(f"W_ih_{d}", [4 * H, H], f32, kind="ExternalInput")
             for d in ("f", "b")}
    fcW_d = nc.dram_tensor("fc_W", [2 * H, EMB], f32r, kind="ExternalInput")
    attnW_d = nc.dram_tensor("attn_W", [EMB, 1], f32r, kind="ExternalInput")
    outW_d = nc.dram_tensor("out_W", [EMB, OUT], f32r, kind="ExternalInput")
    if gcn_bias:
        b_d = [nc.dram_tensor(f"b{i+1}", [c], f32, kind="ExternalInput")
               for i, c in enumerate([64, 128, 256, 256])]
    if lstm_bias:
        bih_d = {d: nc.dram_tensor(f"b_ih_{d}", [4 * H], f32, kind="ExternalInput")
                 for d in ("f", "b")}
        bhh_d = {d: nc.dram_tensor(f"b_hh_{d}", [4 * H], f32, kind="ExternalInput")
                 for d in ("f", "b")}
    if fc_bias:
        fcb_d = nc.dram_tensor("fc_b", [EMB], f32r, kind="ExternalInput")
    if out_bias:
        outb_d = nc.dram_tensor("out_b", [OUT], f32r, kind="ExternalInput")
    out_d = nc.dram_tensor("out", [1, OUT], f32, kind="ExternalOutput")

    with tile.TileContext(nc) as tc:
        # ================= persistent constants =================
        with tc.tile_pool(name="const", bufs=1) as cp:
            AT_r = cp.tile([128, 512], f32r)           # col k*256+d ; A^T[s,d], s=k*128+p
            AT_b = cp.tile([128, 512], bf16)           # same, bf16
            W1blk = cp.tile([128, 128], bf16)          # blockdiag(W1, W1)
            W2blk = cp.tile([128, 256], bf16)          # blockdiag(W2, W2)
            W3_b = cp.tile([128, 256], bf16)           # W3 natural
            W4_b = cp.tile([128, 512], bf16)           # col cc*256 + co
            WihT_sb = [cp.tile([128, 2048], bf16, name=f"WihT{k}") for k in (0, 1)]
            pooledT_bf = cp.tile([128, 64], bf16)
            fcW_sb = cp.tile([128, 1024], f32r)        # col k*256+m
            fcW_bf = cp.tile([128, 1024], bf16)        # bf16 copy for tail mms
            outW_sb = cp.tile([128, 1024], f32r)       # col m*512+o
            outW_bf = cp.tile([128, 1024], bf16)
            attnW_sb = cp.tile([128, 2], f32r)         # col m
            attnW_bf = cp.tile([128, 2], bf16)
            ones_col = cp.tile([128, 1], f32r)
            ones_11 = cp.tile([1, 1], f32r)
            ident = cp.tile([128, 128], f32)
            pooledT_sb = cp.tile([128, 64], f32r)      # col mo*32 + t ; sum relu(z4)/N
            x0_all = cp.tile([128, NPAIR * 256], f32r)  # col j*256+k*128+g*64+c
            x1_all = cp.tile([128, NPAIR * 256], bf16)  # col j*256+m*128+g*64+co
            x2_all = cp.tile([128, NPAIR * 512], bf16)  # col j*512+m*256+g*128+co
            x3_all = cp.tile([128, T * 512], bf16)      # col t*512+m*256+co
            if gcn_bias:
                bb1 = cp.tile([128, 128], bf16)        # col g*64+c  -> b1[c]
                bb2 = cp.tile([128, 256], bf16)        # col g*128+c -> b2[c]
                bb3 = cp.tile([128, 256], bf16)        # col c      -> b3[c]
                b4col_sb = cp.tile([128, 2], f32)      # scaled by 1/N
            if lstm_bias:
                lbias_sb = cp.tile([1, 2048], f32r)    # col gate*512+dir*256+h
            if fc_bias:
                fcb_row = cp.tile([1, EMB], f32r)
                fcb_col = cp.tile([128, 2], f32)
            if out_bias:
                outb_row = cp.tile([1, OUT], f32r)

            ones_f = cp.tile([128, 1], f32)
            scr11 = cp.tile([1, 1], f32)
            nc.gpsimd.memset(ones_f[:], 1.0)
            nc.vector.tensor_copy(ones_col[:], ones_f[:])
            nc.vector.tensor_copy(ones_11[:], ones_f[0:1, :])
            make_identity(nc, ident[:])
            # pre-warm the ACT sqrt table while DMAs are in flight
            nc.scalar.sqrt(scr11[:], ones_f[0:1, :])

            # ---- edge DMA first (A-build is the critical path) ----
            eg_i = cp.tile([128, 64], i32)   # col j<32: src ; col 32+j: dst
            nc.sync.dma_start(
                out=eg_i[:].rearrange("p (two j) -> p two j", two=2),
                in_=edge_d.ap().rearrange("two (p j) -> p two j", p=128))

            # ---- input prefetch ----
            dv = data_d.ap().rearrange("(j two) (k p) c -> two k p j c", two=2, p=128)
            xv = x0_all[:].rearrange("p (j k g c) -> k g p j c", j=NPAIR, k=2, g=2)
            for k in (0, 1):
                for g in (0, 1):
                    nc.sync.dma_start(out=xv[k, g], in_=dv[g, k])

            # ---- weight loads (natural layouts) ----
            w1s = cp.tile([64, 64], f32)
            w2s = cp.tile([64, 128], f32)
            w3s = cp.tile([128, 256], f32)
            w4s = cp.tile([128, 512], f32)
            wnat = [cp.tile([128, 2048], f32, name=f"wnat{k}") for k in (0, 1)]
            nc.sync.dma_start(out=w1s[:], in_=W_d[0].ap())
            nc.sync.dma_start(out=w2s[:], in_=W_d[1].ap())
            nc.sync.dma_start(out=w3s[:], in_=W_d[2].ap())
            nc.sync.dma_start(
                out=w4s[:].rearrange("p (k co) -> p k co", k=2),
                in_=W_d[3].ap().rearrange("(k p) co -> p k co", p=128))
            for di, d in enumerate(("f", "b")):
                nc.sync.dma_start(
                    out=wnat[di][:].rearrange("p (i c) -> p i c", i=8),
                    in_=Wih_d[d].ap().rearrange("(i p) c -> p i c", p=128))
            nc.sync.dma_start(
                out=fcW_sb[:].rearrange("p (k m) -> p k m", k=4),
                in_=fcW_d.ap().rearrange("(k p) m -> p k m", p=128))
            nc.sync.dma_start(
                out=outW_sb[:].rearrange("p (m o) -> p m o", m=2),
                in_=outW_d.ap().rearrange("(m p) o -> p m o", p=128))
            nc.sync.dma_start(
                out=attnW_sb[:].unsqueeze(2),
                in_=attnW_d.ap().rearrange("(m p) one -> p m one", p=128))
            if fc_bias:
                nc.sync.dma_start(out=fcb_row[:], in_=fcb_d.ap().rearrange("m -> 1 m"))
                nc.sync.dma_start(
                    out=fcb_col[:].unsqueeze(2),
                    in_=fcb_d.ap().rearrange("(m p) -> p m", p=128).unsqueeze(2))
            if out_bias:
                nc.sync.dma_start(out=outb_row[:], in_=outb_d.ap().rearrange("o -> 1 o"))

            # blockdiag weight prep on ACT (keep DVE free for one-hots)
            nc.gpsimd.memset(W1blk[:], 0.0)
            nc.gpsimd.memset(W2blk[:], 0.0)
            for g in (0, 1):
                nc.scalar.copy(W1blk[g * 64:(g + 1) * 64, g * 64:(g + 1) * 64],
                               w1s[:])
                nc.scalar.copy(W2blk[g * 64:(g + 1) * 64, g * 128:(g + 1) * 128],
                               w2s[:])
            nc.scalar.copy(W3_b[:], w3s[:])
            nc.scalar.copy(W4_b[:], w4s[:])
            nc.scalar.copy(fcW_bf[:], fcW_sb[:].bitcast(f32))
            nc.scalar.copy(attnW_bf[:], attnW_sb[:].bitcast(f32))
            nc.scalar.copy(outW_bf[:], outW_sb[:].bitcast(f32))

            # ============ stage 0: A^T build ============
            with (
                tc.tile_pool(name="ab_sb", bufs=2) as ab,
                tc.tile_pool(name="ab_ps", bufs=1, space="PSUM") as abp,
                tc.tile_pool(name="oh", bufs=6) as ohp,
            ):
                # iota 0..255 on all 128 partitions directly (no broadcast dep)
                iota_i = ab.tile([128, 256], i32)
                nc.gpsimd.iota(iota_i[:], pattern=[[1, 256]], base=0,
                               channel_multiplier=0)
                iota_bc = ab.tile([128, 256], bf16)
                nc.vector.tensor_copy(iota_bc[:], iota_i[:])

                # edge index columns (DMA'd into eg_i early), cast to f32
                eg_b = ab.tile([128, 64], f32)
                nc.vector.tensor_copy(eg_b[:], eg_i[:])

                # self-loop columns p+128k
                sl_i = ab.tile([128, 2], i32)
                nc.gpsimd.iota(sl_i[:], pattern=[[128, 2]], base=0,
                               channel_multiplier=1)
                sl_b = ab.tile([128, 2], f32)
                nc.vector.tensor_copy(sl_b[:], sl_i[:])

                # accumulate A^T_unnorm = sum_e onehot_src^T(slice) @ onehot_dst
                atun_ps = abp.tile([128, 512], f32)
                for c in range(34):
                    if c < 32:
                        scol = eg_b[:, c:c + 1]
                        dcol = eg_b[:, 32 + c:33 + c]
                    else:
                        scol = dcol = sl_b[:, c - 32:c - 31]
                    oh_s = ohp.tile([128, 256], bf16, tag="ohs")
                    nc.vector.tensor_scalar(oh_s[:], iota_bc[:], scol, None,
                                            op0=ALU.is_equal)
                    if c < 32:
                        oh_d = ohp.tile([128, 256], bf16, tag="ohd")
                        nc.vector.tensor_scalar(oh_d[:], iota_bc[:], dcol, None,
                                                op0=ALU.is_equal)
                    else:
                        oh_d = oh_s
                    for m in (0, 1):
                        nc.tensor.matmul(atun_ps[:, m * 256:(m + 1) * 256],
                                         oh_s[:, m * 128:(m + 1) * 128], oh_d[:],
                                         start=(c == 0 and m == 0),
                                         stop=(c == 33 and m == 1))
                atun_sb = ab.tile([128, 512], f32r)
                nc.scalar.copy(atun_sb[:], atun_ps[:])

                # deg (in-degree, row form), dinv = 1/sqrt(deg)  (deg >= 1 always)
                deg_ps = abp.tile([1, 256], f32, tag="deg")
                for m in (0, 1):
                    nc.tensor.matmul(deg_ps[:], rf(ones_col[:]),
                                     rf(atun_sb[:, m * 256:(m + 1) * 256]),
                                     start=(m == 0), stop=(m == 1))
                deg_sb = ab.tile([1, 256], f32)
                nc.scalar.copy(deg_sb[:], deg_ps[:])
                # column form via PE transpose of the deg row; rsqrt on [128,2]
                degc_ps = abp.tile([128, 2], f32, tag="degc")
                for dm in (0, 1):
                    nc.tensor.transpose(degc_ps[:, dm:dm + 1],
                                        deg_sb[0:1, dm * 128:(dm + 1) * 128],
                                        ident[0:1, 0:1])
                dinv_col = ab.tile([128, 2], f32)
                nc.vector.reciprocal(dinv_col[:], degc_ps[:])
                nc.scalar.sqrt(dinv_col[:], dinv_col[:])
                # row form back via transposes of dinv_col, then broadcast
                dinvr_ps = abp.tile([1, 256], f32, tag="dinvr")
                for dm in (0, 1):
                    nc.tensor.transpose(dinvr_ps[0:1, dm * 128:(dm + 1) * 128],
                                        dinv_col[:, dm:dm + 1],
                                        ident[:])
                dinvr_sb = ab.tile([1, 256], f32)
                nc.scalar.copy(dinvr_sb[:], dinvr_ps[:])
                dinv_bc = ab.tile([128, 256], f32)
                nc.gpsimd.partition_broadcast(dinv_bc[:], dinvr_sb[:])

                # AT_norm[s,d] = dinv[s] * ATun[s,d] * dinv[d]
                for m in (0, 1):
                    nc.vector.scalar_tensor_tensor(
                        out=AT_r[:, m * 256:(m + 1) * 256],
                        in0=atun_sb[:, m * 256:(m + 1) * 256],
                        scalar=dinv_col[:, m:m + 1],
                        in1=dinv_bc[:],
                        op0=ALU.mult, op1=ALU.mult)
                nc.scalar.copy(AT_b[:], rf(AT_r[:]))

                # ---- bias prep ----
                if gcn_bias:
                    b1r = ab.tile([1, 64], f32, tag="b1r")
                    nc.sync.dma_start(out=b1r[:], in_=b_d[0].ap().rearrange("c -> 1 c"))
                    brow = ab.tile([1, 128], f32, tag="brow")
                    nc.vector.tensor_copy(
                        brow[:].rearrange("one (r c) -> one r c", r=2),
                        b1r[:].rearrange("one c -> one 1 c").broadcast_to([1, 2, 64]))
                    nc.gpsimd.partition_broadcast(bb1[:], brow[:])
                    b2r = ab.tile([1, 128], f32, tag="b2r")
                    nc.sync.dma_start(out=b2r[:], in_=b_d[1].ap().rearrange("c -> 1 c"))
                    brow2 = ab.tile([1, 256], f32, tag="brow2")
                    nc.vector.tensor_copy(
                        brow2[:].rearrange("one (r c) -> one r c", r=2),
                        b2r[:].rearrange("one c -> one 1 c").broadcast_to([1, 2, 128]))
                    nc.gpsimd.partition_broadcast(bb2[:], brow2[:])
                    brow3 = ab.tile([1, 256], f32, tag="brow3")
                    nc.sync.dma_start(out=brow3[:],
                                      in_=b_d[2].ap().rearrange("c -> 1 c"))
                    nc.gpsimd.partition_broadcast(bb3[:], brow3[:])
                    b4tmp = ab.tile([128, 2], f32, tag="b4tmp")
                    nc.sync.dma_start(
                        out=b4tmp[:].unsqueeze(2),
                        in_=b_d[3].ap().rearrange("(m p) -> p m", p=128).unsqueeze(2))
                    nc.vector.tensor_scalar_mul(b4col_sb[:], b4tmp[:], 1.0 / N)
                if lstm_bias:
                    for di, d in enumerate(("f", "b")):
                        bi = ab.tile([1, 1024], f32, tag="lbias_i")
                        bh = ab.tile([1, 1024], f32, tag="lbias_h")
                        nc.sync.dma_start(out=bi[:], in_=bih_d[d].ap().rearrange("g -> 1 g"))
                        nc.sync.dma_start(out=bh[:], in_=bhh_d[d].ap().rearrange("g -> 1 g"))
                        nc.vector.tensor_add(
                            lbias_sb[:, di * 256:].rearrange("one (g q) -> one g q", g=4)[:, :, 0:256],
                            bi[:].rearrange("one (g q) -> one g q", g=4),
                            bh[:].rearrange("one (g q) -> one g q", g=4))

            # ================= main GCN loop (layer sweeps) =================
            with (
                tc.tile_pool(name="work", bufs=4) as wk,
                tc.tile_pool(name="ps", bufs=1, space="PSUM") as ps,
            ):
                # ---- L1 sweep (fp32r) ----
                for j in range(NPAIR):
                    xb = j * 256
                    agg1 = ps.tile([128, 256], f32, tag="agg", bufs=3)
                    for k in (0, 1):
                        nc.tensor.matmul(
                            agg1[:], x0_all[:, xb + k * 128: xb + (k + 1) * 128],
                            AT_r[:, k * 256:(k + 1) * 256],
                            start=(k == 0), stop=(k == 1))
                    agg1_sb = wk.tile([128, 256], bf16, tag="agg1sb")
                    nc.scalar.copy(agg1_sb[:], agg1[:])
                    z1full = ps.tile([128, 512], f32, tag="z", bufs=3)
                    for m in (0, 1):
                        nc.tensor.matmul(
                            z1full[:, m * 128:(m + 1) * 128],
                            agg1_sb[:, m * 128:(m + 1) * 128], W1blk[:],
                            start=True, stop=True)
                    x1v = x1_all[:, xb:xb + 256]
                    if gcn_bias:
                        zt = wk.tile([128, 256], bf16, tag="zt1")
                        nc.vector.tensor_add(
                            zt[:].rearrange("p (m q) -> p m q", m=2),
                            z1full[:, 0:256].rearrange("p (m q) -> p m q", m=2),
                            bb1[:].rearrange("p q -> p 1 q").broadcast_to([128, 2, 128]))
                        nc.vector.tensor_relu(x1v, zt[:])
                    else:
                        nc.vector.tensor_relu(x1v, z1full[:, 0:256])

                # ---- W_ih transpose (fills the dinv-chain PE gap) ----
                for di in (0, 1):
                    for ks in (0, 1):
                        wt_ps = ps.tile([128, 512], f32, tag="z", bufs=3)
                        wt_ps2 = ps.tile([128, 512], f32, tag="z", bufs=3)
                        for i in range(8):
                            tgt = wt_ps if i < 4 else wt_ps2
                            nc.tensor.transpose(
                                tgt[:, (i % 4) * 128:(i % 4 + 1) * 128],
                                wnat[di][:, i * 256 + ks * 128:
                                          i * 256 + (ks + 1) * 128],
                                ident[:])
                        # evac with (gate,hh) regrouping; no 1/N (folded in L4)
                        dst = WihT_sb[ks][:].rearrange(
                            "p (g d hh r) -> p g d hh r", g=4, d=2, hh=2)[:, :, di]
                        src1 = wt_ps[:].rearrange("p (g hh r) -> p g hh r",
                                                  g=2, hh=2)
                        src2 = wt_ps2[:].rearrange("p (g hh r) -> p g hh r",
                                                   g=2, hh=2)
                        if di == 0:
                            nc.vector.tensor_copy(dst[:, 0:2], src1)
                            nc.vector.tensor_copy(dst[:, 2:4], src2)
                        else:
                            nc.scalar.copy(dst[:, 0:2], src1)
                            nc.scalar.copy(dst[:, 2:4], src2)

                # ---- L2 sweep (bf16) ----
                for j in range(NPAIR):
                    xb = j * 256
                    agg2 = ps.tile([128, 256], f32, tag="agg", bufs=3)
                    for k in (0, 1):
                        nc.tensor.matmul(
                            agg2[:], x1_all[:, xb + k * 128: xb + (k + 1) * 128],
                            AT_b[:, k * 256:(k + 1) * 256],
                            start=(k == 0), stop=(k == 1))
                    agg2_sb = wk.tile([128, 256], bf16, tag="agg2sb")
                    nc.scalar.copy(agg2_sb[:], agg2[:])
                    z2 = ps.tile([128, 512], f32, tag="z", bufs=3)
                    for m in (0, 1):
                        nc.tensor.matmul(
                            z2[:, m * 256:(m + 1) * 256],
                            agg2_sb[:, m * 128:(m + 1) * 128], W2blk[:],
                            start=True, stop=True)
                    x2v = x2_all[:, j * 512:(j + 1) * 512]
                    if gcn_bias:
                        zt = wk.tile([128, 512], bf16, tag="zt2")
                        nc.vector.tensor_add(
                            zt[:].rearrange("p (m q) -> p m q", m=2),
                            z2[:].rearrange("p (m q) -> p m q", m=2),
                            bb2[:].rearrange("p q -> p 1 q").broadcast_to([128, 2, 256]))
                        nc.vector.tensor_relu(x2v, zt[:])
                    else:
                        nc.vector.tensor_relu(x2v, z2[:])

                # ---- L3 sweep (bf16, per-graph) ----
                for j in range(NPAIR):
                    for g in (0, 1):
                        agg3 = ps.tile([128, 256], f32, tag="agg", bufs=3)
                        for k in (0, 1):
                            nc.tensor.matmul(
                                agg3[:],
                                x2_all[:, j * 512 + k * 256 + g * 128:
                                       j * 512 + k * 256 + (g + 1) * 128],
                                AT_b[:, k * 256:(k + 1) * 256],
                                start=(k == 0), stop=(k == 1))
                        agg3_sb = wk.tile([128, 256], bf16, tag="agg3sb")
                        nc.scalar.copy(agg3_sb[:], agg3[:])
                        z3 = ps.tile([128, 512], f32, tag="z", bufs=3)
                        for m in (0, 1):
                            nc.tensor.matmul(
                                z3[:, m * 256:(m + 1) * 256],
                                agg3_sb[:, m * 128:(m + 1) * 128], W3_b[:],
                                start=True, stop=True)
                        t_idx = 2 * j + g
                        x3v = x3_all[:, t_idx * 512:(t_idx + 1) * 512]
                        if gcn_bias:
                            zt = wk.tile([128, 512], bf16, tag="zt3")
                            nc.vector.tensor_add(
                                zt[:].rearrange("p (m q) -> p m q", m=2),
                                z3[:].rearrange("p (m q) -> p m q", m=2),
                                bb3[:].rearrange("p q -> p 1 q").broadcast_to([128, 2, 256]))
                            nc.vector.tensor_relu(x3v, zt[:])
                        else:
                            nc.vector.tensor_relu(x3v, z3[:])

                # ---- L4 sweep (bf16, per-graph; relu/N + accum -> pooledT) ----
                for j in range(NPAIR):
                    for g in (0, 1):
                        t_idx = 2 * j + g
                        xb3 = t_idx * 512
                        agg4_sb = wk.tile([128, 512], bf16, tag="agg4sb")
                        agg4 = ps.tile([128, 512], f32, tag="z", bufs=3)
                        for cc in (0, 1):
                            for k in (0, 1):
                                nc.tensor.matmul(
                                    agg4[:, cc * 256:(cc + 1) * 256],
                                    x3_all[:, xb3 + k * 256 + cc * 128:
                                           xb3 + k * 256 + (cc + 1) * 128],
                                    AT_b[:, k * 256:(k + 1) * 256],
                                    start=(k == 0), stop=(k == 1))
                        nc.vector.tensor_copy(agg4_sb[:], agg4[:])
                        for mo in (0, 1):
                            z4 = ps.tile([128, 256], f32, tag="z4", bufs=2)
                            for cc in (0, 1):
                                nc.tensor.matmul(
                                    z4[:],
                                    W4_b[:, cc * 256 + mo * 128: cc * 256 + (mo + 1) * 128],
                                    agg4_sb[:, cc * 256:(cc + 1) * 256],
                                    start=(cc == 0), stop=(cc == 1))
                            x4scr = wk.tile([128, 256], bf16, tag="x4scr")
                            ctx_lp = nc.allow_low_precision(
                                reason="fp32r accum (32-bit)")
                            ctx_lp.__enter__()
                            nc.scalar.activation(
                                x4scr[:], z4[:], AF.Relu, scale=1.0 / N,
                                bias=(b4col_sb[:, mo:mo + 1] if gcn_bias
                                      else 0.0),
                                accum_out=pooledT_sb[:, mo * 32 + t_idx:
                                                     mo * 32 + t_idx + 1])
                            ctx_lp.__exit__(None, None, None)

                # pre-warm the ACT tanh table before the tail needs it
                nc.scalar.activation(scr11[:], ones_f[0:1, :], AF.Tanh)

            # ================= LSTM + fc + attention + head =================
            with (
                tc.tile_pool(name="tail", bufs=1) as tl,
                tc.tile_pool(name="tailps_g", bufs=1, space="PSUM") as tpg,
                tc.tile_pool(name="tailps", bufs=2, space="PSUM") as tp,
            ):
                # gates TRANSPOSED: gT [128 r, cj*32+t], cj = gate*4 + dir*2 + hh
                # (gate j = gate*512 + dir*256 + hh*128 + r).  f-gate (cj 4..7)
                # is skipped: with zero init state it never reaches the output.
                nc.vector.tensor_copy(pooledT_bf[:], rf(pooledT_sb[:]))
                gT = tpg.tile([128, 512], f32, tag="gates")
                if lstm_bias:
                    lb_colT_ps = tp.tile([128, 16], f32, tag="small")
                    for cj in range(16):
                        nc.tensor.transpose(
                            lb_colT_ps[:, cj:cj + 1],
                            rf(lbias_sb[0:1, cj * 128:(cj + 1) * 128]),
                            ident[0:1, 0:1])
                    lb_colT = tl.tile([128, 16], f32)
                    nc.vector.tensor_copy(lb_colT[:], lb_colT_ps[:])
                    # i/o gates go through tanh(x/2): bias must be pre-halved
                    nc.vector.tensor_scalar_mul(lb_colT[:, 0:4],
                                                lb_colT_ps[:, 0:4], 0.5)
                    nc.vector.tensor_scalar_mul(lb_colT[:, 12:16],
                                                lb_colT_ps[:, 12:16], 0.5)
                for cj in [0, 1, 2, 3, 8, 9, 10, 11, 12, 13, 14, 15]:
                    for k in (0, 1):
                        nc.tensor.matmul(
                            gT[:, cj * 32:(cj + 1) * 32],
                            WihT_sb[k][:, cj * 128:(cj + 1) * 128],
                            pooledT_bf[:, k * 32:(k + 1) * 32],
                            start=(k == 0), stop=(k == 1))
                # sigmoid(x) = 0.5*tanh(x/2)+0.5 -> single Tanh table load
                tanh_g = tl.tile([128, 128], f32)
                th_i = tl.tile([128, 128], f32)
                th_o = tl.tile([128, 128], f32)
                if lstm_bias:
                    for cj in range(4):
                        nc.scalar.activation(
                            tanh_g[:, cj * 32:(cj + 1) * 32],
                            gT[:, 256 + cj * 32: 256 + (cj + 1) * 32],
                            AF.Tanh, bias=lb_colT[:, 8 + cj:9 + cj])
                        nc.scalar.activation(
                            th_i[:, cj * 32:(cj + 1) * 32],
                            gT[:, cj * 32:(cj + 1) * 32],
                            AF.Tanh, scale=0.5,
                            bias=lb_colT[:, cj:cj + 1])
                        nc.scalar.activation(
                            th_o[:, cj * 32:(cj + 1) * 32],
                            gT[:, 384 + cj * 32: 384 + (cj + 1) * 32],
                            AF.Tanh, scale=0.5,
                            bias=lb_colT[:, 12 + cj:13 + cj])
                else:
                    nc.scalar.activation(tanh_g[:], gT[:, 256:384], AF.Tanh)
                    nc.scalar.activation(th_i[:], gT[:, 0:128], AF.Tanh, scale=0.5)
                    nc.scalar.activation(th_o[:], gT[:, 384:512], AF.Tanh, scale=0.5)
                sig_i = tl.tile([128, 128], f32)
                sig_o = tl.tile([128, 128], f32)
                nc.vector.tensor_scalar(sig_i[:], th_i[:], 0.5, 0.5,
                                        op0=ALU.mult, op1=ALU.add)
                nc.vector.tensor_scalar(sig_o[:], th_o[:], 0.5, 0.5,
                                        op0=ALU.mult, op1=ALU.add)
                c_sb = tl.tile([128, 128], f32)
                nc.vector.tensor_mul(c_sb[:], sig_i[:], tanh_g[:])
                tc_sb = tl.tile([128, 128], f32)
                nc.scalar.activation(tc_sb[:], c_sb[:], AF.Tanh)
                hT_sb = tl.tile([128, 128], bf16)
                nc.vector.tensor_mul(hT_sb[:], sig_o[:], tc_sb[:])

                # emb (node-major) [32, 256]
                emb_ps = tp.tile([32, 256], f32, tag="small")
                for k in range(4):
                    nc.tensor.matmul(emb_ps[:], hT_sb[:, k * 32:(k + 1) * 32],
                                     fcW_bf[:, k * 256:(k + 1) * 256],
                                     start=(k == 0), stop=(k == 3))
                if fc_bias:
                    fcb_bc = tl.tile([32, 256], f32)
                    nc.gpsimd.partition_broadcast(fcb_bc[:], rf(fcb_row[:]))
                    nc.vector.tensor_add(emb_ps[:], emb_ps[:], fcb_bc[:])
                emb_sb = tl.tile([32, 256], bf16)
                nc.vector.tensor_copy(emb_sb[:], emb_ps[:])

                # embT [128, (mo,t)]
                embT_ps = tp.tile([128, 64], f32, tag="small")
                for mo in (0, 1):
                    for k in range(4):
                        nc.tensor.matmul(
                            embT_ps[:, mo * 32:(mo + 1) * 32],
                            fcW_bf[:, k * 256 + mo * 128: k * 256 + (mo + 1) * 128],
                            hT_sb[:, k * 32:(k + 1) * 32],
                            start=(k == 0), stop=(k == 3))
                embT_sb = tl.tile([128, 64], bf16)
                if fc_bias:
                    for mo in (0, 1):
                        nc.scalar.activation(embT_sb[:, mo * 32:(mo + 1) * 32],
                                             embT_ps[:, mo * 32:(mo + 1) * 32],
                                             AF.Identity,
                                             bias=fcb_col[:, mo:mo + 1])
                else:
                    nc.vector.tensor_copy(embT_sb[:], embT_ps[:])

                # attention scores [1, 32] ; softmax over free dim
                sc_ps = tp.tile([1, 32], f32, tag="small")
                for mo in (0, 1):
                    nc.tensor.matmul(sc_ps[:], attnW_bf[:, mo:mo + 1],
                                     embT_sb[:, mo * 32:(mo + 1) * 32],
                                     start=(mo == 0), stop=(mo == 1))
                sc_sb = tl.tile([1, 32], f32)
                nc.vector.tensor_copy(sc_sb[:], sc_ps[:])
                # exp(x) = (1+tanh(x/2))/(1-tanh(x/2)); scores are O(1), no
                # max-subtraction needed, and the tanh table is already loaded
                tht = tl.tile([1, 32], f32)
                nc.scalar.activation(tht[:], sc_sb[:], AF.Tanh, scale=0.5)
                num = tl.tile([1, 32], f32)
                den = tl.tile([1, 32], f32)
                nc.vector.tensor_scalar(num[:], tht[:], 1.0, 1.0,
                                        op0=ALU.mult, op1=ALU.add)
                nc.vector.tensor_scalar(den[:], tht[:], -1.0, 1.0,
                                        op0=ALU.mult, op1=ALU.add)
                nc.vector.reciprocal(den[:], den[:])
                ex = tl.tile([1, 32], f32)
                nc.vector.tensor_mul(ex[:], num[:], den[:])
                ssum = tl.tile([1, 1], f32)
                nc.vector.tensor_reduce(ssum[:], ex[:], axis=mybir.AxisListType.X,
                                        op=ALU.add)
                rs = tl.tile([1, 1], f32)
                nc.vector.reciprocal(rs[:], ssum[:])
                w_row = tl.tile([1, 32], f32r)
                nc.vector.tensor_scalar_mul(w_row[:], ex[:], rs[:])

                # w column; xwc[m,1] = sum_t emb[t,m] * w[t]  (direct, no xw row)
                wc_ps = tp.tile([32, 1], f32, tag="small")
                nc.tensor.matmul(wc_ps[:], rf(w_row[:]), rf(ones_11[:]),
                                 start=True, stop=True)
                wc_sb = tl.tile([32, 1], bf16)
                nc.vector.tensor_copy(wc_sb[:], wc_ps[:])
                xwc_ps = tp.tile([128, 2], f32, tag="small")
                for mo in (0, 1):
                    nc.tensor.matmul(xwc_ps[:, mo:mo + 1],
                                     emb_sb[:, mo * 128:(mo + 1) * 128],
                                     wc_sb[:], start=True, stop=True)
                xwc_bf = tl.tile([128, 2], bf16)
                nc.vector.tensor_copy(xwc_bf[:], xwc_ps[:])
                fin_ps = tp.tile([1, 512], f32, tag="small")
                for mo in (0, 1):
                    nc.tensor.matmul(fin_ps[:], xwc_bf[:, mo:mo + 1],
                                     outW_bf[:, mo * 512:(mo + 1) * 512],
                                     start=(mo == 0), stop=(mo == 1 and not out_bias))
                if out_bias:
                    nc.tensor.matmul(fin_ps[:], rf(ones_11[:]), rf(outb_row[:]),
                                     start=False, stop=True)
                fin_sb = tl.tile([1, 512], f32)
                nc.vector.tensor_copy(fin_sb[:], fin_ps[:])
                nc.sync.dma_start(out=out_d.ap(), in_=fin_sb[:])

    nc.compile()
    return nc


def _get_nc(flags):
    key = tuple(sorted(flags.items()))
    if key not in _CACHE:
        _CACHE[key] = _build(flags)
    return _CACHE[key]


def kernel(**inputs):
    from concourse import bass_utils

    inp = {k: np.asarray(v) for k, v in inputs.items()}
    flags = {
        "gcn_bias": any(np.any(inp[f"b{i}"]) for i in (1, 2, 3, 4)),
        "lstm_bias": any(np.any(inp[k]) for k in
                         ("b_ih_f", "b_hh_f", "b_ih_b", "b_hh_b")),
        "fc_bias": bool(np.any(inp["fc_b"])),
        "out_bias": bool(np.any(inp["out_b"])),
    }
    nc = _get_nc(flags)

    base = {
        "edge_index": np.ascontiguousarray(inp["edge_index"].astype(np.int32)),
        "W1": np.ascontiguousarray(inp["W1"].astype(np.float32)),
        "W2": np.ascontiguousarray(inp["W2"].astype(np.float32)),
        "W3": np.ascontiguousarray(inp["W3"].astype(np.float32)),
        "W4": np.ascontiguousarray(inp["W4"].astype(np.float32)),
        "W_ih_f": np.ascontiguousarray(inp["W_ih_f"].astype(np.float32)),
        "W_ih_b": np.ascontiguousarray(inp["W_ih_b"].astype(np.float32)),
        "fc_W": np.ascontiguousarray(inp["fc_W"].astype(np.float32)),
        "attn_W": np.ascontiguousarray(inp["attn_W"].astype(np.float32)),
        "out_W": np.ascontiguousarray(inp["out_W"].astype(np.float32)),
    }
    if flags["gcn_bias"]:
        for i in (1, 2, 3, 4):
            base[f"b{i}"] = np.ascontiguousarray(inp[f"b{i}"].astype(np.float32))
    if flags["lstm_bias"]:
        for k in ("b_ih_f", "b_hh_f", "b_ih_b", "b_hh_b"):
            base[k] = np.ascontiguousarray(inp[k].astype(np.float32))
    if flags["fc_bias"]:
        base["fc_b"] = np.ascontiguousarray(inp["fc_b"].astype(np.float32))
    if flags["out_bias"]:
        base["out_b"] = np.ascontiguousarray(inp["out_b"].astype(np.float32))

    data = inp["data"].astype(np.float32)
    in_maps = [dict(base, data_local=np.ascontiguousarray(data[c]))
               for c in range(NCORES)]

    global LAST_RESULT
    res = bass_utils.run_bass_kernel_spmd(nc, in_maps,
                                          core_ids=list(range(NCORES)),
                                          **RUN_KWARGS)
    LAST_RESULT = res
    return np.concatenate([r["out"] for r in res.results], axis=0)


if __name__ == "__main__":
    import reference
    inputs = {k: np.asarray(v) for k, v in reference.setup_inputs().items()}
    got = kernel(**inputs)
    print(got.shape, got.dtype)
